# revision 1
# baseline (speedup 1.0000x reference)
"""Trainium2 Bass kernel for the BIMM2D mixture NLL loss (nn_BIMM2D_test_11441792876621).

Strategy (data-parallel over 8 NeuronCores, M axis sharded):
  The loss is rewritten as nll = S0 - mean_m ln p[m] with
    p[m] = sum_{c in pos|int} exp(arg[m,c]) - sum_{c in neg} exp(arg[m,c])
    arg[m,c] = sum_r feat[r,m] * coef[r,c]
  feats = {u, v, ln v, q' = 0.5 u^2/sigma_n^2 + v^2/sn2, 1}   (computed on device)
  coef  = constant [5, 1540] matrix derived from the MC samples / params (host,
          O(n_int*N) work only). Columns: 768 interface-pos, 4 interior, 768
          interface-neg terms; interface columns absorb log w_j - log N; S0=20
          keeps every exp within fp32 range (validated: args in [-173, 22]).

  Per 128-m tile on each core: one K=27 bf16 matmul group (3x512 + 1x4 free)
  computes all 1540 args into PSUM (3-way bf16 splits of data and coef rows
  reproduce fp32-accurate products), then two ScalarE Exp ops with accum_out
  produce the (interior+pos) and neg sums directly. Finale: p = acc_a - acc_b,
  Ln, reduce.
  Per-core partial sums of ln p are combined on host: one scalar per core.
"""
import math
import sys

import numpy as np

sys.path.insert(0, "/opt/trn_rl_repo")

import ml_dtypes  # noqa: E402

LOG_GAMMA_3_2 = math.log(math.gamma(1.5))
S0 = 20.0
N_COL = 1540
SPLIT_PAIRS = [(0, 0), (0, 1), (0, 2), (1, 0), (1, 1), (2, 0)]  # (data_i, coef_j)
NCORES = 8

_ERF = np.vectorize(math.erf, otypes=[np.float64])


def _bf16_split3(x):
    x = np.asarray(x, np.float32)
    d1 = x.astype(ml_dtypes.bfloat16).astype(np.float32)
    r1 = (x - d1).astype(np.float32)
    d2 = r1.astype(ml_dtypes.bfloat16).astype(np.float32)
    r2 = (r1 - d2).astype(np.float32)
    d3 = r2.astype(ml_dtypes.bfloat16).astype(np.float32)
    return [d1, d2, d3]


def _host_constants(uniform_eps, I, W, sigma_b, sigma_n, d, r):
    n_phases = I.shape[0]
    n_int, N = uniform_eps.shape
    rho = np.tanh(np.float64(r))
    sn2 = np.float64(sigma_n) ** 2 * (1.0 - rho)
    sig_eff = np.float64(sigma_n) * np.sqrt(1.0 - rho)
    sn_sq = np.float64(sigma_n) ** 2
    logW = np.asarray(W, np.float64)
    log_w = logW - (np.log(np.sum(np.exp(logW - logW.max()))) + logW.max())
    ia, ib = np.triu_indices(n_phases, k=1)
    CONST = (-np.log(np.float64(sigma_n)) - 0.5 * np.log(2 * np.pi)
             - 0.5 * np.log(sn2) - 0.5 * np.log(np.pi))

    pos = np.zeros((5, n_int * N))
    for j in range(n_int):
        Ia, Ib = np.float64(I[ia[j]]), np.float64(I[ib[j]])
        eps = np.asarray(uniform_eps[j], np.float64)
        ux = eps * 2.0 * np.float64(d) * np.float64(sigma_b) - np.float64(d) * np.float64(sigma_b)
        x = ux / (np.sqrt(2.0) * np.float64(sigma_b))
        In = (_ERF(x) + 1.0) * 0.5 * (Ib - Ia) + Ia
        G = (Ib - Ia) / np.sqrt(2.0 * np.pi * np.float64(sigma_b) ** 2) * np.exp(-(x ** 2))
        s = slice(j * N, (j + 1) * N)
        pos[0, s] = In / sn_sq
        pos[1, s] = 2.0 * G / sn2
        pos[2, s] = 1.0
        pos[3, s] = -1.0
        pos[4, s] = (CONST - np.log(G) - 0.5 * In ** 2 / sn_sq - G ** 2 / sn2
                     + log_w[n_phases + j] - np.log(N) + S0)
    neg = pos.copy()
    neg[1] = -neg[1]

    intr = np.zeros((5, n_phases))
    beta_int_const = (np.log(2.0) - LOG_GAMMA_3_2 - 3.0 * np.log(sig_eff)
                      - np.log(np.float64(sigma_n)) - 0.5 * np.log(2 * np.pi))
    for i in range(n_phases):
        intr[0, i] = np.float64(I[i]) / sn_sq
        intr[2, i] = 2.0
        intr[3, i] = -1.0
        intr[4, i] = beta_int_const + log_w[i] - 0.5 * np.float64(I[i]) ** 2 / sn_sq + S0

    coef = np.concatenate([intr, pos, neg], axis=1)
    assert coef.shape == (5, N_COL)
    return coef, sn_sq, sn2


def _build_coef_input(coef):
    rows = []
    for f in range(4):
        sp = _bf16_split3(coef[f])
        for (_, cj) in SPLIT_PAIRS:
            rows.append(sp[cj])
    sp = _bf16_split3(coef[4])
    rows += [sp[0], sp[1], sp[2]]
    block = np.stack(rows)
    out = np.zeros((128, N_COL), np.float32)
    for g in range(4):
        out[32 * g:32 * g + 27] = block
    return out.astype(ml_dtypes.bfloat16)


def _build_core_kernel_v1(nc, M_core, sn_sq, sn2, repeat=1):
    import concourse.bass as bass
    import concourse.tile as tile
    from concourse import mybir

    F32 = mybir.dt.float32
    BF16 = mybir.dt.bfloat16
    EXP = mybir.ActivationFunctionType.Exp
    LN = mybir.ActivationFunctionType.Ln
    SQUARE = mybir.ActivationFunctionType.Square
    ADD = mybir.AluOpType.add
    SUB = mybir.AluOpType.subtract

    W = M_core // 128
    FG = M_core // 4
    TPG = FG // 128
    T = M_core // 128

    if repeat < 0:  # null kernel: I/O only (for overhead calibration)
        import concourse.tile as tile2
        d_u0 = nc.dram_tensor("u", [M_core], F32, kind="ExternalInput")
        d_v0 = nc.dram_tensor("v", [M_core], F32, kind="ExternalInput")
        d_c0 = nc.dram_tensor("coef", [128, N_COL], BF16, kind="ExternalInput")
        d_o0 = nc.dram_tensor("out", [1, 1], F32, kind="ExternalOutput")
        with tile2.TileContext(nc) as tc0:
            with tc0.tile_pool(name="nul", bufs=1) as nul:
                t0 = nul.tile([1, 1], F32)
                nc.vector.memset(t0, 0.0)
                nc.gpsimd.dma_start(out=d_o0[:, :], in_=t0)
        return nc

    d_u = nc.dram_tensor("u", [M_core], F32, kind="ExternalInput")
    d_v = nc.dram_tensor("v", [M_core], F32, kind="ExternalInput")
    d_coef = nc.dram_tensor("coef", [128, N_COL], BF16, kind="ExternalInput")
    d_out = nc.dram_tensor("out", [1, 1], F32, kind="ExternalOutput")

    inv_sqrt2_sn = float(1.0 / math.sqrt(2.0 * sn_sq))
    inv_sqrt_sn2 = float(1.0 / math.sqrt(sn2))

    with tile.TileContext(nc) as tc:
        with tc.tile_pool(name="const", bufs=1) as constp, \
             tc.tile_pool(name="prep", bufs=1) as prep, \
             tc.tile_pool(name="packp", bufs=1) as packp, \
             tc.tile_pool(name="eout", bufs=3) as eoutp, \
             tc.tile_pool(name="fin", bufs=1) as finp, \
             tc.tile_pool(name="ps", bufs=2, space="PSUM") as psp:

            coef_sb = constp.tile([128, N_COL], BF16)
            nc.gpsimd.dma_start(out=coef_sb, in_=d_coef[:, :])

            pack = packp.tile([128, FG], BF16)
            nc.vector.memset(pack, 0.0)

            u2d = prep.tile([128, W], F32)
            v2d = prep.tile([128, W], F32)
            nc.gpsimd.dma_start(out=u2d, in_=d_u[:].rearrange("(p w) -> p w", w=W))
            nc.gpsimd.dma_start(out=v2d, in_=d_v[:].rearrange("(p w) -> p w", w=W))
            lv = prep.tile([128, W], F32)
            nc.scalar.activation(out=lv, in_=v2d, func=LN)
            s1 = prep.tile([128, W], F32)
            nc.scalar.activation(out=s1, in_=u2d, func=SQUARE, scale=inv_sqrt2_sn)
            s2 = prep.tile([128, W], F32)
            nc.scalar.activation(out=s2, in_=v2d, func=SQUARE, scale=inv_sqrt_sn2)
            qp = prep.tile([128, W], F32)
            nc.vector.tensor_tensor(out=qp, in0=s1, in1=s2, op=ADD)

            ones_st = prep.tile([128, W], BF16)
            nc.vector.memset(ones_st, 1.0)

            dma_engines = [nc.gpsimd, nc.sync, nc.scalar]
            n_dma = 0

            def scatter(row, src_ap):
                nonlocal n_dma
                dma_engines[n_dma % 3].dma_start(out=pack[row:128:32, :], in_=src_ap)
                n_dma += 1

            for fi, feat in enumerate([u2d, v2d, lv, qp]):
                d1 = prep.tile([128, W], BF16, tag=f"d1_{fi}")
                nc.vector.tensor_copy(out=d1, in_=feat)
                r1 = prep.tile([128, W], F32, tag=f"r1_{fi}")
                nc.vector.tensor_tensor(out=r1, in0=feat, in1=d1, op=SUB)
                d2 = prep.tile([128, W], BF16, tag=f"d2_{fi}")
                nc.vector.tensor_copy(out=d2, in_=r1)
                r2 = prep.tile([128, W], F32, tag=f"r2_{fi}")
                nc.vector.tensor_tensor(out=r2, in0=r1, in1=d2, op=SUB)
                d3 = prep.tile([128, W], BF16, tag=f"d3_{fi}")
                nc.vector.tensor_copy(out=d3, in_=r2)
                splits = [d1, d2, d3]
                for slot, (di, _) in enumerate(SPLIT_PAIRS):
                    scatter(fi * 6 + slot, splits[di][:, :])
            for rr in range(3):
                scatter(24 + rr, ones_st[:, :])

            acc_a = finp.tile([128, T], F32)
            acc_b = finp.tile([128, T], F32)
            if repeat == 0:
                nc.vector.memset(acc_a, 1.0)
                nc.vector.memset(acc_b, 0.5)

            for rep in range(repeat):
                for t in range(T):
                    g, i = divmod(t, TPG)
                    lhsT = pack[32 * g:32 * g + 27, 128 * i:128 * (i + 1)]
                    rhs_base = 32 * g
                    psum = psp.tile([128, 2048], F32, tag="args")
                    tp = (32 * g, 0)
                    for blk in range(3):
                        nc.tensor.matmul(
                            out=psum[:, 512 * blk:512 * (blk + 1)],
                            lhsT=lhsT,
                            rhs=coef_sb[rhs_base:rhs_base + 27, 512 * blk:512 * (blk + 1)],
                            start=True, stop=True, tile_position=tp)
                    nc.tensor.matmul(
                        out=psum[:, 1536:1540],
                        lhsT=lhsT,
                        rhs=coef_sb[rhs_base:rhs_base + 27, 1536:1540],
                        start=True, stop=True, tile_position=tp)
                    e_all = eoutp.tile([128, 1540], F32, tag="e_all")
                    nc.scalar.activation(out=e_all, in_=psum[:, 0:1540], func=EXP,
                                         accum_out=acc_a[:, t:t + 1])
                    nc.vector.tensor_reduce(out=acc_b[:, t:t + 1],
                                            in_=e_all[:, 772:1540], op=ADD,
                                            axis=mybir.AxisListType.X)

            # p = acc_a - 2*acc_b  (acc_a = pos+int+neg sum, acc_b = neg sum)
            two_b = finp.tile([128, T], F32)
            nc.vector.tensor_tensor(out=two_b, in0=acc_b, in1=acc_b, op=ADD)
            p_all = finp.tile([128, T], F32)
            nc.vector.tensor_tensor(out=p_all, in0=acc_a, in1=two_b, op=SUB)
            lnp = finp.tile([128, T], F32)
            nc.scalar.activation(out=lnp, in_=p_all, func=LN)
            rsum = finp.tile([128, 1], F32)
            nc.vector.tensor_reduce(out=rsum, in_=lnp, op=ADD,
                                    axis=mybir.AxisListType.X)
            row = finp.tile([1, 128], F32)
            nc.gpsimd.dma_start(out=row, in_=rsum[:, :])
            total = finp.tile([1, 1], F32)
            nc.vector.tensor_reduce(out=total, in_=row, op=ADD,
                                    axis=mybir.AxisListType.X)
            nc.gpsimd.dma_start(out=d_out[:, :], in_=total)
    return nc




_build_core_kernel = _build_core_kernel_v1


def kernel(u, v, uniform_eps, I, W, sigma_b, sigma_n, d, r):
    import jax
    import concourse.bacc as bacc
    from concourse.bass_utils import run_bass_kernel_spmd

    platforms = {dev.platform for dev in jax.devices()}
    if platforms == {"cpu"}:
        raise RuntimeError("No neuron/axon devices visible to JAX")

    u = np.asarray(u, np.float32)
    v = np.asarray(v, np.float32)
    M = u.shape[0]
    MC = M // NCORES

    coef, sn_sq, sn2 = _host_constants(
        np.asarray(uniform_eps), np.asarray(I), np.asarray(W),
        np.asarray(sigma_b), np.asarray(sigma_n), np.asarray(d), np.asarray(r))
    coef_in = _build_coef_input(coef)

    nc = bacc.Bacc()
    _build_core_kernel(nc, MC, sn_sq, sn2)
    nc.finalize()

    in_maps = [{"u": u[c * MC:(c + 1) * MC], "v": v[c * MC:(c + 1) * MC],
                "coef": coef_in} for c in range(NCORES)]
    res = run_bass_kernel_spmd(nc, in_maps, list(range(NCORES)))
    total = sum(float(res.results[c]["out"][0, 0]) for c in range(NCORES))
    nll = S0 - total / M
    return np.float32(nll)

